# revision 41
# baseline (speedup 1.0000x reference)
"""Block-causal attention TRN2 kernel (8-core SPMD, head-sharded), v2.

Problem: y = (softmax(mask(Q K^T / sqrt(d))) V) W_out + b_out where
Q,K,V = x W_qkv + b_qkv, x [2, 2048, 1024], 16 heads of d=64, block-causal
mask with chunk 128.

Sharding: core c handles batch b = c//4 and head group g = c%4 (4 heads).
Each core computes its heads' QKV projection (W_qkv column slice), the
block-causal attention, and a partial out-projection against its W_out row
slice. The host sums the 4 partial outputs per batch and adds b_out.

v2 layout (all matmul operands bf16, f32 accumulation in PSUM):
 - x^T is produced by XBAR DMA transposes (one [128,1024] instruction per
   128-row strip) after a gpsimd f32->bf16 cast, keeping the tensor engine
   and DVE out of the transpose path entirely.
 - Q^T/K^T [d, l] come from the projection with W stationary; V is
   projected in key-major orientation directly (x^T tiles stationary, W_v
   moving), so V needs no transpose. The V bias rides a K=1 ones-row
   matmul in the same accumulation group.
 - Per head pair, both heads' score tiles land in one two-bank PSUM tile
   and a single fused activation computes exp for both (scale folded in),
   writing bf16. V carries interleaved ones columns so attn@V also
   accumulates the softmax denominators.
 - Normalization: DVE reciprocal of the denominator rows, gpsimd
   partition_broadcast (idle Pool engine), DVE multiply into o^T bf16.
 - Attention processes both head pairs interleaved every j-tile (one
   two-bank accumulator each), with next-tile projection matmuls drained
   one per j-step to fill the exp-latency gaps; PSUM = 2 (proj/outproj)
   + 2 (scores) + 4 (accumulators) banks.

W_qkv/W_out are pre-cast to bf16 on the host (weights; b_qkv stays f32
for the q/k bias add).
"""

import sys

for _p in ("/opt/trn_rl_repo", "/root/.axon_site/_ro/trn_rl_repo"):
    if _p not in sys.path:
        sys.path.append(_p)

import numpy as np
import ml_dtypes

import concourse.bass as bass
import concourse.mybir as mybir
import concourse.tile as tile
from concourse import bacc
from concourse.bass_utils import run_bass_kernel_spmd

F32 = mybir.dt.float32
BF16 = mybir.dt.bfloat16
EXP = mybir.ActivationFunctionType.Exp
ADD = mybir.AluOpType.add
MULT = mybir.AluOpType.mult

B, L, D = 2, 2048, 1024
H, DH = 16, 64          # total heads, head dim
CHUNK = 128
HPC = 4                 # heads per core
S = HPC * DH            # 256 per-core qkv width per projection
N_CORES = 8
LT = 512                # l-tile (i-tile) size
NLT = L // LT           # 4
NKT = D // 128          # 8 k-tiles over D
NJT = L // CHUNK        # 16 j-tiles/chunks
SCALE = 1.0 / float(np.sqrt(DH))
VW = 130                # v_sb columns per head pair: [V_h0|ones|ones|V_h1]


def build_program(repeat=1):
    nc = bacc.Bacc("TRN2", target_bir_lowering=False, debug=False)
    x_d = nc.dram_tensor("x", [L, D], F32, kind="ExternalInput")
    w_d = nc.dram_tensor("w_qkv", [D, 3 * S], BF16, kind="ExternalInput")
    bq_d = nc.dram_tensor("b_qkv", [3 * S], F32, kind="ExternalInput")
    wo_d = nc.dram_tensor("w_out", [S, D], BF16, kind="ExternalInput")
    y_d = nc.dram_tensor("y", [L, D], F32, kind="ExternalOutput")

    with tile.TileContext(nc) as tc:
        lp = nc.allow_low_precision(reason="bf16 matmul pipeline")
        lp.__enter__()
        with tc.tile_pool(name="const", bufs=1) as const, \
             tc.tile_pool(name="big", bufs=1) as big, \
             tc.tile_pool(name="stage", bufs=8) as stage, \
             tc.tile_pool(name="xbp", bufs=8) as xbp, \
             tc.tile_pool(name="xtp", bufs=2) as xtp, \
             tc.tile_pool(name="expp", bufs=6) as expp, \
             tc.tile_pool(name="work", bufs=3) as work, \
             tc.tile_pool(name="small", bufs=2) as small, \
             tc.tile_pool(name="ps_pp", bufs=2, space="PSUM") as ps_pp, \
             tc.tile_pool(name="ps_s", bufs=1, space="PSUM") as ps_s, \
             tc.tile_pool(name="ps_o", bufs=2, space="PSUM") as ps_o:

            # ---- constants ----
            F32R = mybir.dt.float32r
            ones64 = const.tile([1, 64], F32R)
            nc.vector.memset(ones64[:].bitcast(F32), 1.0)
            # selector [1,128]: 0 for rows 0:64, 1 for rows 64:128
            sel128 = const.tile([1, 128], F32R)
            nc.vector.memset(sel128[:, 0:64].bitcast(F32), 0.0)
            nc.vector.memset(sel128[:, 64:128].bitcast(F32), 1.0)
            # q/k biases as per-partition columns [128, 4] (q p0, q p1, k p0, k p1)
            bq_sb = const.tile([128, 4], F32)
            bq_ap = bq_d.ap()
            nc.sync.dma_start(
                out=bq_sb[:],
                in_=bass.AP(tensor=bq_ap.tensor, offset=bq_ap.offset,
                            ap=[[1, 128], [128, 4]]),
            )
            # v bias as a bf16 row [1, 256]
            ones1 = const.tile([1, 128], BF16)
            nc.vector.memset(ones1[:], 1.0)
            bv_f = const.tile([1, S], F32)
            nc.sync.dma_start(
                out=bv_f[:],
                in_=bass.AP(tensor=bq_ap.tensor, offset=bq_ap.offset + 2 * S,
                            ap=[[S, 1], [1, S]]))
            bv_row = const.tile([1, S], BF16)
            nc.vector.tensor_copy(bv_row[:], bv_f[:])

            # ---- persistent weights/activations ----
            w_sb = big.tile([128, NKT, 3 * S], BF16)       # W_qkv k-tiles
            w_r = w_d.ap().rearrange("(kt p) c -> p kt c", p=128)
            for c0, c1 in ((0, 256), (256, 512), (512, 768)):
                nc.sync.dma_start(out=w_sb[:, :, c0:c1], in_=w_r[:, :, c0:c1])
            wo_sb = big.tile([128, 2, D], BF16)            # W_out k-tiles (head pairs)
            for p in range(2):
                nc.sync.dma_start(out=wo_sb[:, p, :],
                                  in_=wo_d[p * 128:(p + 1) * 128, :])
            qt_sb = big.tile([128, 2, L], BF16)            # Q^T pair-stacked
            kt_sb = big.tile([128, 2, L], BF16)            # K^T pair-stacked
            v_sb = big.tile([128, NJT, 2 * VW], BF16)      # V + ones columns
            ot_sb = big.tile([128, 2, L], BF16)            # normalized o^T

            def init_v_const():
                # ones at cols p*VW + hh*65 + 64 ([V_h|ones] per head)
                vap = v_sb[:]
                o4 = bass.AP(tensor=vap.tensor, offset=vap.offset + 64,
                             ap=list(vap.ap[:2]) + [[VW, 2], [65, 2]])
                nc.vector.memset(o4, 1.0)

            def fetch_xs(rep, t):
                tiles = []
                for sp in range(4):
                    xst = stage.tile([128, D], F32, tag="xs",
                                     name=f"rxs_{rep}_{t}_{sp}")
                    nc.sync.dma_start(
                        out=xst[:],
                        in_=x_d[t * LT + sp * 128: t * LT + (sp + 1) * 128, :])
                    tiles.append(xst)
                return tiles

            def emit_xT(rep, t, xs):
                """bf16 cast (Pool) + XBAR DMA transpose -> xT [128, kt, LT]."""
                xT = xtp.tile([128, NKT, LT], BF16, tag="xT",
                              name=f"rxT_{rep}_{t}")
                for sp in range(4):
                    xb = xbp.tile([128, D], BF16, tag="xb",
                                  name=f"rxb_{rep}_{t}_{sp}")
                    nc.gpsimd.tensor_copy(xb[:], xs[sp][:])
                    nc.sync.dma_start_transpose(
                        out=xT[:, :, sp * 128:(sp + 1) * 128], in_=xb[:])
                return xT

            def emit_proj(rep, t, xT):
                """QKV projection closures for l-tile t (4 q/k + 4 v units)."""
                l0 = t * LT
                units = []
                for ct in range(4):
                    def u(ct=ct, xT=xT, rep=rep, t=t, l0=l0):
                        isq = ct < 2
                        p = ct % 2
                        wc = (0 if isq else 256) + p * 128
                        pp = ps_pp.tile([128, LT], F32, tag="pp",
                                        name=f"rpp_{rep}_{t}_{ct}")
                        for kt in range(NKT):
                            nc.tensor.matmul(
                                pp[:], w_sb[:, kt, wc:wc + 128], xT[:, kt, :],
                                start=(kt == 0), stop=(kt == NKT - 1))
                        dst = qt_sb if isq else kt_sb
                        nc.vector.tensor_scalar(
                            out=dst[:, p, l0:l0 + LT], in0=pp[:],
                            scalar1=bq_sb[:, ct:ct + 1], scalar2=None, op0=ADD)
                    units.append(u)
                for lb in range(4):
                    def u(lb=lb, xT=xT, rep=rep, t=t):
                        jt = 4 * t + lb
                        vp = ps_pp.tile([128, LT], F32, tag="pp",
                                        name=f"rvp_{rep}_{t}_{lb}")
                        nc.tensor.matmul(vp[:, 0:S], ones1[:], bv_row[:],
                                         start=True, stop=False)
                        for kt in range(NKT):
                            nc.tensor.matmul(
                                vp[:, 0:S],
                                xT[:, kt, lb * 128:(lb + 1) * 128],
                                w_sb[:, kt, 2 * S:3 * S],
                                start=False, stop=(kt == NKT - 1))
                        # scatter heads into [V_h0|ones|V_h1|ones] pair blocks
                        vap = v_sb[:, jt, :]
                        dst = bass.AP(
                            tensor=vap.tensor, offset=vap.offset,
                            ap=list(vap.ap[:1]) + [[VW, 2], [65, 2], [1, 64]])
                        src = vp[:, 0:S].rearrange("q (p h d) -> q p h d",
                                                   p=2, h=2)
                        nc.vector.tensor_copy(dst, src)
                    units.append(u)
                return units

            def emit_outproj(rep, t):
                """Out-projection closures for i-tile t (8 units)."""
                units = []
                last = (t == NLT - 1)
                for st in range(4):
                    for mt in range(2):
                        def u(st=st, mt=mt, rep=rep, t=t, last=last):
                            i0 = t * LT + st * 128
                            yp = ps_pp.tile([128, 512], F32, tag="pp",
                                            name=f"ryp_{rep}_{t}_{st}_{mt}")
                            for p in range(2):
                                nc.tensor.matmul(
                                    yp[:], ot_sb[:, p, i0:i0 + 128],
                                    wo_sb[:, p, mt * 512:(mt + 1) * 512],
                                    start=(p == 0), stop=(p == 1))
                            y_sb = work.tile([128, 512], F32, tag="y_sb",
                                             name=f"rysb_{rep}_{t}_{st}_{mt}")
                            if last:
                                nc.scalar.copy(y_sb[:], yp[:])
                            else:
                                nc.vector.tensor_copy(y_sb[:], yp[:])
                            nc.sync.dma_start(
                                out=y_d[i0:i0 + 128, mt * 512:(mt + 1) * 512],
                                in_=y_sb[:])
                        units.append(u)
                return units

            def attn_scores(rep, t, p, jt):
                """mm_s pair + fused exp; returns the bf16 exp tile."""
                l0 = t * LT
                vis = max(0, jt - 4 * t) * 128
                s2 = ps_s.tile([128, 2, LT], F32, tag="s",
                               name=f"rs_{p}_{rep}_{t}_{jt}")
                for hh in range(2):
                    nc.tensor.matmul(
                        s2[:, hh, vis:LT],
                        kt_sb[hh * 64:(hh + 1) * 64, p,
                              jt * 128:(jt + 1) * 128],
                        qt_sb[hh * 64:(hh + 1) * 64, p, l0 + vis:l0 + LT],
                        start=True, stop=True)
                e2 = expp.tile([128, 2, LT], BF16, tag="e_t",
                               name=f"re_{p}_{rep}_{t}_{jt}")
                nc.scalar.activation(e2[:, :, vis:LT], s2[:, :, vis:LT],
                                     EXP, scale=SCALE)
                return e2

            def attn_av(rep, t, p, jt, e2, o_ps):
                njt = 4 * (t + 1)
                vis = max(0, jt - 4 * t) * 128
                for hh in range(2):
                    c0 = p * VW + hh * 65
                    nc.tensor.matmul(
                        o_ps[0:65, hh, vis:LT],
                        v_sb[:, jt, c0:c0 + 65],
                        e2[:, hh, vis:LT],
                        start=(jt == 0), stop=(jt == njt - 1))

            def attn_recip(rep, t, p, o_ps):
                """DVE reciprocals of both denominator rows for pair p."""
                r2 = small.tile([1, 2, LT], F32R, tag="r2",
                                name=f"rr2_{p}_{rep}_{t}")
                nc.vector.reciprocal(r2[:, 0, :], o_ps[64:65, 0, :])
                nc.vector.reciprocal(r2[:, 1, :], o_ps[64:65, 1, :])
                return r2

            def attn_normalize_mul(rep, t, p, o_ps, r2):
                """Reciprocal rows broadcast across partitions via K=1 PE
                matmuls (gpsimd partition_broadcast is sim-only on this
                runtime); rbp rides the double-buffered pp pool so both
                pairs' chains overlap."""
                l0 = t * LT
                rbp = ps_pp.tile([128, LT], F32, tag="pp",
                                 name=f"rrbp_{p}_{rep}_{t}")
                nc.tensor.matmul(rbp[:], sel128[:], r2[:, 1, :],
                                 start=True, stop=True)
                nc.tensor.matmul(rbp[0:64, :], ones64[:], r2[:, 0, :],
                                 start=True, stop=True)
                rb = work.tile([128, LT], F32, tag="rb",
                               name=f"rrb_{p}_{rep}_{t}")
                nc.scalar.copy(rb[:], rbp[:])
                nc.vector.tensor_mul(
                    ot_sb[0:64, p, l0:l0 + LT], o_ps[0:64, 0, :], rb[0:64, :])
                nc.vector.tensor_mul(
                    ot_sb[64:128, p, l0:l0 + LT], o_ps[0:64, 1, :],
                    rb[64:128, :])

            def emit_attention(rep, t, fillers, late_fillers=None):
                """Both pairs' j-loops zipped; fillers spread evenly across
                the j-steps (attention alone is ACT-bound, so PE idles
                without them). late_fillers overwrite state this tile still
                reads in its first 4 j-steps, so they only drain from jt>=4.
                """
                o_ps = [ps_o.tile([128, 2, LT], F32, tag="o_ps",
                                  name=f"ro_{p}_{rep}_{t}")
                        for p in range(2)]
                njt = 4 * (t + 1)
                late = list(late_fillers) if late_fillers else []

                def drain(half):
                    # slightly under-drain so ~2 units remain at loop end to
                    # cover the normalize reciprocal latency
                    want = (total * half) // (2 * njt + 2)
                    while drained[0] < want and fillers:
                        fillers.pop(0)()
                        drained[0] += 1
                    if half > 8 and late:
                        lwant = (len_late * (half - 8)
                                 + 2 * (njt - 4) - 1) // (2 * (njt - 4))
                        while ldrained[0] < lwant and late:
                            late.pop(0)()
                            ldrained[0] += 1

                drained = [0]
                ldrained = [0]
                total = len(fillers)
                len_late = len(late)
                pend = None
                half = 0
                for jt in range(njt):
                    for p in range(2):
                        e2 = attn_scores(rep, t, p, jt)
                        half += 1
                        drain(half)
                        if pend is not None:
                            attn_av(rep, t, *pend)
                        pend = (p, jt, e2, o_ps[p])
                if pend is not None:
                    attn_av(rep, t, *pend)
                r2s = [attn_recip(rep, t, p, o_ps[p]) for p in range(2)]
                for u in fillers:
                    u()
                fillers.clear()
                for u in late:
                    u()
                for p in range(2):
                    attn_normalize_mul(rep, t, p, o_ps[p], r2s[p])

            xs_next = fetch_xs(0, 0)
            a_emitted = False
            for rep in range(repeat):
                pending = []
                for t in range(NLT):
                    if not a_emitted:
                        xT = emit_xT(rep, t, xs_next)
                        u2 = emit_proj(rep, t, xT)
                        u2[0]()
                        if rep == 0:
                            init_v_const()
                        for u in u2[1:]:
                            u()
                    # next tile's projection units fill this tile's stalls
                    if t < NLT - 1:
                        xs_next = fetch_xs(rep, t + 1)
                        xTn = emit_xT(rep, t + 1, xs_next)
                        pending.extend(emit_proj(rep, t + 1, xTn))
                        a_emitted = True
                        emit_attention(rep, t, pending)
                        pending.extend(emit_outproj(rep, t))
                    else:
                        # last tile: next rep's projection units drain as
                        # gated fillers once this tile's reads of the l<512
                        # slices are past (jt>=4); out-proj follows the
                        # normalize chain.
                        late = []
                        if rep < repeat - 1:
                            xs_next = fetch_xs(rep + 1, 0)
                            xTn = emit_xT(rep + 1, 0, xs_next)
                            late = emit_proj(rep + 1, 0, xTn)
                            a_emitted = True
                        else:
                            a_emitted = False
                        emit_attention(rep, t, pending, late)
                        for u in emit_outproj(rep, t):
                            u()
        lp.__exit__(None, None, None)
    nc.compile()
    return nc


_NC_CACHE = {}


def _get_nc():
    if "nc" not in _NC_CACHE:
        _NC_CACHE["nc"] = build_program()
    return _NC_CACHE["nc"]


def make_in_maps(x, W_qkv, b_qkv, W_out):
    x = np.ascontiguousarray(np.asarray(x, dtype=np.float32))
    W_qkv = np.asarray(W_qkv, dtype=np.float32)
    b_qkv = np.asarray(b_qkv, dtype=np.float32)
    W_out = np.asarray(W_out, dtype=np.float32)
    in_maps = []
    for c in range(N_CORES):
        b, g = divmod(c, 4)
        cols = np.concatenate([np.arange(blk * D + g * S, blk * D + (g + 1) * S)
                               for blk in range(3)])
        in_maps.append({
            "x": np.ascontiguousarray(x[b]),
            "w_qkv": np.ascontiguousarray(
                W_qkv[:, cols].astype(ml_dtypes.bfloat16)),
            "b_qkv": np.ascontiguousarray(b_qkv[cols]),
            "w_out": np.ascontiguousarray(
                W_out[g * S:(g + 1) * S, :].astype(ml_dtypes.bfloat16)),
        })
    return in_maps


def kernel(x, W_qkv, b_qkv, W_out, b_out):
    nc = _get_nc()
    in_maps = make_in_maps(x, W_qkv, b_qkv, W_out)
    res = run_bass_kernel_spmd(nc, in_maps, list(range(N_CORES)))
    b_out = np.asarray(b_out, dtype=np.float32)
    out = np.zeros((B, L, D), dtype=np.float32)
    for c in range(N_CORES):
        out[c // 4] += res.results[c]["y"]
    out += b_out[None, None, :]
    return out
